# revision 37
# baseline (speedup 1.0000x reference)
"""GCN layer (PyG GCNConv + ReLU) on 8 Trainium2 NeuronCores.

emb = D^-1/2 (A+I) D^-1/2 (x @ W) + b ; returns (emb, relu(emb))

Strategy (aggregate-then-transform, dst-sharded):
  emb = (A_norm @ x) @ W + b  -- algebraically identical, 8x less matmul work
  - Nodes (dst) sharded across 8 cores: core c owns rows [c*12544, (c+1)*12544).
  - x replicated (bf16) in each core's HBM; per-edge rows fetched with the
    dma_gather custom op (256 B/row). int16 gather indices limit the source
    window to <32768 rows, so x is split into 4 chunks of 25088 rows.
  - PAIR PACKING: adjacent dst blocks (2b, 2b+1) share one padded slot run
    per chunk. Block 2b's edges fill forward from slot 0, block 2b+1's fill
    backward from the end; the boundary tile can hold edges of both blocks
    (their onehot columns mask each other out). This cuts gather padding from
    ~25% to ~13%, and gather descriptor generation on gpsimd is the kernel's
    rate limiter (~2 ns/desc with 4-queue overlap).
  - Scatter-add realized as PE matmul: for each 128-edge tile,
    aggT[fi, d] += G[e, fi]^T @ onehot[e, d], accumulated in PSUM over a
    128-dst-node block window.
  - Onehots for a whole block (all tiles incl. the self-loop tile) are built
    with TWO wide DVE tensor_tensor ops over [128, til*128] using broadcast
    access patterns whose innermost dim is a duplicated (stride 1, count 2)
    pair -- keeps the DVE 2x perf mode eligible.
  - Self-loop is a regular tile: G = own x rows (contiguous DMA), onehot =
    diag(selfnorm) from the same batched build.
  - Stage 2 per block: emb = aggT^T @ W + b (bias seeded via K=1 outer
    product), pipelined one block behind aggregation; outputs staged bf16.
  - Groups of 4 pairs (8 blocks) pipeline gather->onehot->matmul->stage2; the
    final group is a single pair so the drain tail is short.

Host does the O(E) graph prep with numpy; a pure-numpy emulation of the
device layout (_host_verify) can be enabled with GCN_HOST_VERIFY=1.
"""

import os
import numpy as np
import ml_dtypes

import concourse.bass as bass
import concourse.tile as tile
from concourse import bacc, mybir
from concourse.bass_utils import run_bass_kernel_spmd

P = 128            # partitions / tile edge
F = 128            # feature dim (in == out)
NC = 8             # cores
N = 100000         # nodes (full problem)
BLOCKS_PER_CORE = 98
NCHUNK = 4         # src chunks (int16 index range)
PAIRS_PER_GROUP = 4

BF16 = mybir.dt.bfloat16
F32 = mybir.dt.float32
I16 = mybir.dt.int16

_cache: dict = {}


def _wrap16(flat):
    """flat [n] int16 -> [32, n/16]: idx i at [i%16, i//16], tiled x2.

    dma_gather on queue q reads its indices only from partition band
    [32q, 32q+32) (one 16-partition wrap per Q7 core of the pair), so a
    group's four chunk-calls can share the same columns in different bands.
    """
    n = flat.shape[0]
    a = flat.reshape(n // 16, 16).T            # [16, n/16]
    return np.tile(a, (2, 1))


def _host_prep(x, W, b, edge_index, edge_weight, n_nodes, blocks_per_core,
               n_cores, n_chunks=NCHUNK):
    """Build per-core input maps. Returns (in_maps, layout)."""
    p = P
    npc = blocks_per_core * p
    n_pad = n_cores * npc
    cs = n_pad // n_chunks          # chunk rows
    assert cs < 32768
    n_blocks = n_cores * blocks_per_core
    npair = blocks_per_core // 2

    src = edge_index[0].astype(np.int64)
    dst = edge_index[1].astype(np.int64)
    w = edge_weight.astype(np.float64)

    deg = np.bincount(dst, weights=w, minlength=n_nodes) + 1.0  # + self-loop
    dinv = 1.0 / np.sqrt(deg)
    norm = dinv[src] * w * dinv[dst]

    blk = dst // p                            # global dst block
    chunk = src // cs                         # src chunk
    seg = blk * n_chunks + chunk              # segment id
    order = np.argsort(seg, kind="stable")
    n_segs = n_blocks * n_chunks
    cnt = np.bincount(seg[order], minlength=n_segs).reshape(n_blocks, n_chunks)
    starts = np.zeros(n_segs + 1, dtype=np.int64)
    starts[1:] = np.cumsum(cnt.ravel())

    Tq = max(1, int(np.ceil(cnt.max() / p)))
    til = n_chunks * Tq + 1                   # onehot cols per block (+self)

    # per-core counts [cores, bpc, nchunk] -> pair totals -> static T table
    cntc = cnt.reshape(n_cores, blocks_per_core, n_chunks)
    ptot = cntc[:, 0::2, :] + cntc[:, 1::2, :]        # [cores, npair, nchunk]
    T = np.maximum(Tq, np.ceil(ptot.max(axis=0) / p).astype(np.int64))
    # group structure: groups of PAIRS_PER_GROUP pairs with a gradual taper
    # (3,3,2,1) at the end: PE starts a group only after the whole group's
    # gather DMA lands, so small final groups shrink the drain tail, and the
    # gradual cadence step avoids the gather sprinting into the gather-pool
    # buffer cap (which showed as an 11us gpsimd stall at a 4->2 transition)
    groups = []
    i = 0
    while npair - i > 5:
        groups.append(list(range(i, i + PAIRS_PER_GROUP)))
        i += PAIRS_PER_GROUP
    while npair - i > 0:
        take = min(2, npair - i)
        groups.append(list(range(i, i + take)))
        i += take

    # self-loop norm (per global padded node)
    selfnorm = np.zeros(n_pad)
    selfnorm[:n_nodes] = dinv * dinv

    per_edge = {
        "lidx": src - chunk * cs,             # chunk-local src row
        "norm": norm,
        "dstl": dst % p,
    }

    x_pad = np.zeros((n_pad, F), dtype=ml_dtypes.bfloat16)
    x_pad[:n_nodes] = x.astype(ml_dtypes.bfloat16)
    iota = np.broadcast_to(np.arange(p, dtype=np.float64), (p, p))
    iota = np.ascontiguousarray(iota.astype(ml_dtypes.bfloat16))
    w_f32 = np.ascontiguousarray(W.astype(np.float32))
    b_f32 = np.ascontiguousarray(b.astype(np.float32).reshape(1, F))
    ones = np.ones((1, p), dtype=np.float32)

    in_maps = []
    for c in range(n_cores):
        b0 = c * blocks_per_core
        norm_meta = np.zeros((blocks_per_core, til, p))
        dstl_meta = np.zeros((blocks_per_core, til, p))
        norm_meta[:, n_chunks * Tq, :] = \
            selfnorm.reshape(n_blocks, p)[b0:b0 + blocks_per_core]
        dstl_meta[:, n_chunks * Tq, :] = np.arange(p)

        idx_groups = []    # per group: [128, maxcols] int16, band q = call q
        for g_pairs in groups:
            calls = []
            for q in range(n_chunks):
                parts = []
                for pp in g_pairs:
                    bA = 2 * pp
                    bB = 2 * pp + 1
                    S = int(T[pp, q]) * p
                    sl_idx = np.zeros(S, dtype=np.int16)
                    for bb, fwd in ((bA, True), (bB, False)):
                        gb = b0 + bb
                        s0, s1 = starts[gb * n_chunks + q], \
                            starts[gb * n_chunks + q + 1]
                        eid = order[s0:s1]
                        nE = s1 - s0
                        off = 0 if fwd else S - nE
                        sl_idx[off:off + nE] = per_edge["lidx"][eid]
                        # onehot meta columns for this block & chunk
                        nloc = np.zeros(Tq * p)
                        dloc = np.zeros(Tq * p)
                        moff = off if fwd else off - (int(T[pp, q]) - Tq) * p
                        nloc[moff:moff + nE] = per_edge["norm"][eid]
                        dloc[moff:moff + nE] = per_edge["dstl"][eid]
                        norm_meta[bb, q * Tq:(q + 1) * Tq, :] = \
                            nloc.reshape(Tq, p)
                        dstl_meta[bb, q * Tq:(q + 1) * Tq, :] = \
                            dloc.reshape(Tq, p)
                    parts.append(sl_idx)
                calls.append(_wrap16(np.concatenate(parts)))
            maxcols = max(ca.shape[1] for ca in calls)
            arr = np.zeros((P, maxcols), dtype=np.int16)
            for q, ca in enumerate(calls):
                arr[32 * q:32 * (q + 1), :ca.shape[1]] = ca
            idx_groups.append(np.ascontiguousarray(arr))

        def meta_layout(m):
            # [bpc, til, p] -> [p, bpc*til] -> paired duplication (x2 cols)
            flat = m.transpose(2, 0, 1).reshape(p, blocks_per_core * til)
            return np.ascontiguousarray(
                np.repeat(flat, 2, axis=1).astype(ml_dtypes.bfloat16))

        im = {
            "x": x_pad,
            "xself": np.ascontiguousarray(x_pad[c * npc:(c + 1) * npc]),
            "w_in": w_f32,
            "b_in": b_f32,
            "ones_in": ones,
            "iota_in": iota,
            "norm_in": meta_layout(norm_meta),
            "dstl_in": meta_layout(dstl_meta),
        }
        for gi, arr in enumerate(idx_groups):
            im[f"idx{gi}_in"] = arr
        in_maps.append(im)

    layout = {
        "Tq": Tq, "til": til, "T": tuple(map(tuple, T)),
        "groups": tuple(map(tuple, groups)),
        "n_pad": n_pad, "bpc": blocks_per_core, "n_chunks": n_chunks,
    }
    return in_maps, layout


def _host_verify(in_maps, layout, W, b, exp_emb, core=0):
    """Numpy emulation of the device compute path for one core."""
    p = P
    Tq, til = layout["Tq"], layout["til"]
    T = np.array(layout["T"])
    groups = layout["groups"]
    n_chunks = layout["n_chunks"]
    cs = layout["n_pad"] // n_chunks
    bpc = layout["bpc"]
    im = in_maps[core]
    xf = np.asarray(im["x"]).astype(np.float64)
    xself = np.asarray(im["xself"]).astype(np.float64)
    nm = np.asarray(im["norm_in"]).astype(np.float64)[:, ::2]   # unpair
    dm = np.asarray(im["dstl_in"]).astype(np.float64)[:, ::2]
    aggT = np.zeros((bpc, p, F))
    for gi, g_pairs in enumerate(groups):
        idx = np.asarray(im[f"idx{gi}_in"])            # [128, maxcols]
        for q in range(n_chunks):
            ncols = int(T[list(g_pairs), q].sum()) * p // 16
            band = idx[32 * q:32 * q + 16]
            flat = band[:, :ncols].T.reshape(-1)       # unwrap
            poff = 0
            for pp in g_pairs:
                S = int(T[pp, q]) * p
                sl = flat[poff:poff + S].astype(np.int64)
                poff += S
                for half, base in ((0, 0), (1, int(T[pp, q]) - Tq)):
                    bb = 2 * pp + half
                    for t in range(Tq):
                        G = xf[q * cs + sl[(base + t) * p:(base + t + 1) * p]]
                        col = bb * til + q * Tq + t
                        oh = (np.arange(p)[None, :] == dm[:, col][:, None]) \
                            * nm[:, col][:, None]
                        aggT[bb] += oh.T @ G
    errs = []
    for bb in range(bpc):
        sn = nm[:, bb * til + n_chunks * Tq]
        agg = aggT[bb] + xself[bb * p:(bb + 1) * p] * sn[:, None]
        emb = agg @ W.astype(np.float64) + b.astype(np.float64)
        r0 = (core * bpc + bb) * p
        ref = exp_emb[r0:r0 + p]
        if len(ref):
            errs.append(np.abs(emb[:len(ref)] - ref).max())
    print(f"host_verify core {core}: max err {max(errs):.4e}")
    return max(errs)


def _build_program(layout):
    """Emit the SPMD Tile program. Same program runs on every core."""
    p = P
    Tq = layout["Tq"]
    til = layout["til"]
    T = np.array(layout["T"])
    groups = layout["groups"]
    blocks_per_core = layout["bpc"]
    n_chunks = layout["n_chunks"]
    n_pad = layout["n_pad"]
    npc = blocks_per_core * p
    cs = n_pad // n_chunks

    # per-(group, chunk) call sizes in tiles; each call is issued as two
    # half-group gathers (split at a pair boundary) so PE can start on the
    # first half while the second is still in flight
    call_tiles = [[int(T[list(g), q].sum()) for q in range(n_chunks)]
                  for g in groups]
    splits = [(len(g) + 1) // 2 for g in groups]
    tiles_a = [[int(T[list(g[:splits[gi]]), q].sum()) for q in range(n_chunks)]
               for gi, g in enumerate(groups)]
    max_tiles_q = [max(ct[q] for ct in call_tiles) for q in range(n_chunks)]
    max_a_q = [max(ta[q] for ta in tiles_a) for q in range(n_chunks)]
    max_b_q = [max(call_tiles[gi][q] - tiles_a[gi][q]
                   for gi in range(len(groups))) for q in range(n_chunks)]
    max_blocks = max(2 * len(g) for g in groups)

    nc = bacc.Bacc("TRN2", target_bir_lowering=False, debug=False,
                   enable_asserts=False, num_devices=NC,
                   num_swdge_queues=4)

    x_d = nc.dram_tensor("x", [n_pad, F], BF16, kind="ExternalInput")
    xself_d = nc.dram_tensor("xself", [npc, F], BF16, kind="ExternalInput")
    w_d = nc.dram_tensor("w_in", [F, F], F32, kind="ExternalInput")
    b_d = nc.dram_tensor("b_in", [1, F], F32, kind="ExternalInput")
    ones_d = nc.dram_tensor("ones_in", [1, p], F32, kind="ExternalInput")
    iota_d = nc.dram_tensor("iota_in", [p, p], BF16, kind="ExternalInput")
    norm_d = nc.dram_tensor("norm_in", [p, blocks_per_core * til * 2], BF16,
                            kind="ExternalInput")
    dstl_d = nc.dram_tensor("dstl_in", [p, blocks_per_core * til * 2], BF16,
                            kind="ExternalInput")
    idx_d = []
    for gi, ct in enumerate(call_tiles):
        cols = max(ct) * p // 16          # calls stacked in partition bands
        idx_d.append(nc.dram_tensor(f"idx{gi}_in", [p, cols], I16,
                                    kind="ExternalInput"))
    emb_d = nc.dram_tensor("emb_out", [npc, F], BF16, kind="ExternalOutput")
    relu_d = nc.dram_tensor("relu_out", [npc, F], BF16, kind="ExternalOutput")

    emb_v = emb_d.ap().rearrange("(B q) f -> q B f", q=p)    # [p, blocks, F]
    relu_v = relu_d.ap().rearrange("(B q) f -> q B f", q=p)
    xself_v = xself_d.ap().rearrange("(B q) f -> q B f", q=p)

    with tile.TileContext(nc) as tc:
        with (
            tc.tile_pool(name="const", bufs=1) as const_pool,
            tc.tile_pool(name="gather", bufs=3) as gpool,
            tc.tile_pool(name="onehot", bufs=9) as ohpool,
            tc.tile_pool(name="aggsb", bufs=3) as aggpool,
            tc.tile_pool(name="outsb", bufs=2) as outpool,
            tc.tile_pool(name="psum_agg", bufs=3, space="PSUM") as ps_agg,
            tc.tile_pool(name="psum_emb", bufs=3, space="PSUM") as ps_emb,
        ):
            idx_g = []
            for gi, ct in enumerate(call_tiles):
                cols = max(ct) * p // 16
                t = const_pool.tile([p, cols], I16, name=f"idx_g{gi}")
                idx_g.append(t)
            nc.sync.dma_start(out=idx_g[0][:], in_=idx_d[0].ap())
            w_sb = const_pool.tile([F, F], F32)
            nc.sync.dma_start(out=w_sb[:], in_=w_d.ap())
            b_sb = const_pool.tile([1, F], F32)
            nc.sync.dma_start(out=b_sb[:], in_=b_d.ap())
            ones_sb = const_pool.tile([1, p], F32)
            nc.sync.dma_start(out=ones_sb[:], in_=ones_d.ap())
            iota_sb = const_pool.tile([p, p], BF16)
            nc.sync.dma_start(out=iota_sb[:], in_=iota_d.ap())
            norm_sb = const_pool.tile([p, blocks_per_core * til * 2], BF16)
            nc.sync.dma_start(out=norm_sb[:], in_=norm_d.ap())
            dstl_sb = const_pool.tile([p, blocks_per_core * til * 2], BF16)
            nc.sync.dma_start(out=dstl_sb[:], in_=dstl_d.ap())
            for gi in range(1, len(groups)):
                nc.sync.dma_start(out=idx_g[gi][:], in_=idx_d[gi].ap())

            iota_bc = (iota_sb[:].rearrange("q (a f b) -> q a f b", a=1, b=2)
                       .to_broadcast([p, til, p // 2, 2]))

            pending = []     # deferred stage-2 closures (one block behind)

            def flush_pending():
                while pending:
                    pending.pop(0)()

            for gi, g_pairs in enumerate(groups):
                nblk = 2 * len(g_pairs)
                blk0 = 2 * g_pairs[0]
                gq = []
                for q in range(n_chunks):
                    ntile = call_tiles[gi][q]
                    ta = tiles_a[gi][q]
                    tb = ntile - ta
                    gta = gpool.tile([p, max_a_q[q] * F], BF16, tag=f"g{q}a")
                    nc.gpsimd.dma_gather(
                        out_ap=gta[:, :ta * F]
                        .rearrange("q (j f) -> q j f", f=F),
                        in_ap=x_d.ap()[q * cs:(q + 1) * cs, :],
                        idxs_ap=idx_g[gi][:, :ta * p // 16],
                        num_idxs=ta * p,
                        num_idxs_reg=ta * p,
                        elem_size=F,
                        single_packet=False,
                        queue_num=q)
                    gtb = None
                    if tb > 0:
                        gtb = gpool.tile([p, max_b_q[q] * F], BF16,
                                         tag=f"g{q}b")
                        nc.gpsimd.dma_gather(
                            out_ap=gtb[:, :tb * F]
                            .rearrange("q (j f) -> q j f", f=F),
                            in_ap=x_d.ap()[q * cs:(q + 1) * cs, :],
                            idxs_ap=idx_g[gi][:, ta * p // 16:ntile * p // 16],
                            num_idxs=tb * p,
                            num_idxs_reg=tb * p,
                            elem_size=F,
                            single_packet=False,
                            queue_num=q)
                    gq.append((gta, gtb, ta))
                gs = gpool.tile([p, max_blocks * F], BF16, tag="gself")
                nc.sync.dma_start(
                    out=gs[:, :nblk * F].rearrange("q (B f) -> q B f", f=F),
                    in_=xself_v[:, blk0:blk0 + nblk, :])

                emb_st = outpool.tile([p, max_blocks * F], BF16, tag="emb_st")
                relu_st = outpool.tile([p, max_blocks * F], BF16,
                                       tag="relu_st")
                for j, pp in enumerate(g_pairs):
                    # tile offset of this pair within each chunk's call
                    pair_off = [int(T[list(g_pairs[:j]), q].sum())
                                for q in range(n_chunks)]
                    for half in range(2):
                        bb = 2 * pp + half
                        bi = 2 * j + half
                        c0 = bb * til
                        oh = ohpool.tile([p, til * p], BF16, tag="oh")
                        oh_v = oh[:].rearrange("q (u f b) -> q u f b",
                                               f=p // 2, b=2)
                        dstl_bc = (dstl_sb[:, 2 * c0:2 * (c0 + til)]
                                   .rearrange("q (u a b) -> q u a b",
                                              a=1, b=2)
                                   .to_broadcast([p, til, p // 2, 2]))
                        nc.vector.tensor_tensor(
                            out=oh_v, in0=iota_bc, in1=dstl_bc,
                            op=mybir.AluOpType.is_equal)
                        norm_bc = (norm_sb[:, 2 * c0:2 * (c0 + til)]
                                   .rearrange("q (u a b) -> q u a b",
                                              a=1, b=2)
                                   .to_broadcast([p, til, p // 2, 2]))
                        nc.vector.tensor_tensor(
                            out=oh_v, in0=oh_v, in1=norm_bc,
                            op=mybir.AluOpType.mult)

                        agg_ps = ps_agg.tile([p, p], F32)
                        for u in range(til - 1):
                            q, t = divmod(u, Tq)
                            base = pair_off[q] + (0 if half == 0 else
                                                  int(T[pp, q]) - Tq)
                            gta, gtb, ta = gq[q]
                            src = gta if j < splits[gi] else gtb
                            lbase = base if j < splits[gi] else base - ta
                            nc.tensor.matmul(
                                out=agg_ps[:],
                                lhsT=src[:, (lbase + t) * F:
                                         (lbase + t + 1) * F],
                                rhs=oh[:, u * p:(u + 1) * p],
                                start=(u == 0), stop=False)
                        nc.tensor.matmul(
                            out=agg_ps[:],
                            lhsT=gs[:, bi * F:(bi + 1) * F],
                            rhs=oh[:, (til - 1) * p:til * p],
                            start=False, stop=True)

                        agg_sb = aggpool.tile([p, p], F32, tag="agg")
                        nc.scalar.activation(
                            out=agg_sb[:], in_=agg_ps[:],
                            func=mybir.ActivationFunctionType.Copy)

                        def stage2(agg_sb=agg_sb, emb_st=emb_st,
                                   relu_st=relu_st, bi=bi, nblk=nblk,
                                   blk0=blk0):
                            emb_ps = ps_emb.tile([p, F], F32)
                            nc.tensor.matmul(out=emb_ps[:], lhsT=ones_sb[:],
                                             rhs=b_sb[:], start=True,
                                             stop=False)
                            nc.tensor.matmul(out=emb_ps[:], lhsT=agg_sb[:],
                                             rhs=w_sb[:], start=False,
                                             stop=True)
                            nc.scalar.activation(
                                out=emb_st[:, bi * F:(bi + 1) * F],
                                in_=emb_ps[:],
                                func=mybir.ActivationFunctionType.Copy)
                            nc.scalar.activation(
                                out=relu_st[:, bi * F:(bi + 1) * F],
                                in_=emb_ps[:],
                                func=mybir.ActivationFunctionType.Relu)
                            if bi == nblk - 1:
                                nc.sync.dma_start(
                                    out=emb_v[:, blk0:blk0 + nblk, :],
                                    in_=emb_st[:, :nblk * F]
                                    .rearrange("q (B f) -> q B f", f=F))
                                nc.sync.dma_start(
                                    out=relu_v[:, blk0:blk0 + nblk, :],
                                    in_=relu_st[:, :nblk * F]
                                    .rearrange("q (B f) -> q B f", f=F))

                        flush_pending()
                        pending.append(stage2)
            flush_pending()

    nc.compile()
    return nc


def _get_program(layout):
    key = (layout["Tq"], layout["til"], layout["T"], layout["groups"],
           layout["n_pad"], layout["bpc"], layout["n_chunks"])
    if key not in _cache:
        _cache[key] = _build_program(layout)
    return _cache[key]


def run(x, W, b, edge_index, edge_weight, n_nodes, blocks_per_core, n_cores,
        n_chunks=NCHUNK, trace=False):
    in_maps, layout = _host_prep(x, W, b, edge_index, edge_weight,
                                 n_nodes, blocks_per_core, n_cores, n_chunks)
    nc = _get_program(layout)
    res = run_bass_kernel_spmd(nc, in_maps, list(range(n_cores)), trace=trace)
    emb = np.concatenate([np.asarray(res.results[c]["emb_out"])
                          for c in range(n_cores)], axis=0)[:n_nodes]
    relu = np.concatenate([np.asarray(res.results[c]["relu_out"])
                           for c in range(n_cores)], axis=0)[:n_nodes]
    return (emb.astype(np.float32), relu.astype(np.float32)), res


def kernel(x, W, b, level, edge_index, edge_weight):
    x = np.asarray(x)
    W = np.asarray(W)
    b = np.asarray(b)
    edge_index = np.asarray(edge_index)
    edge_weight = np.asarray(edge_weight)
    (emb, relu), _ = run(x, W, b, edge_index, edge_weight,
                         N, BLOCKS_PER_CORE, NC)
    return emb, relu


# revision 38
# speedup vs baseline: 1.1077x; 1.1077x over previous
"""GCN layer (PyG GCNConv + ReLU) on 8 Trainium2 NeuronCores.

emb = D^-1/2 (A+I) D^-1/2 (x @ W) + b ; returns (emb, relu(emb))

Strategy (aggregate-then-transform, dst-sharded):
  emb = (A_norm @ x) @ W + b  -- algebraically identical, 8x less matmul work
  - Nodes (dst) sharded across 8 cores: core c owns rows [c*12544, (c+1)*12544).
  - x replicated (bf16) in each core's HBM; per-edge rows fetched with the
    dma_gather custom op (256 B/row). int16 gather indices limit the source
    window to <32768 rows, so x is split into 4 chunks of 25088 rows.
  - PAIR PACKING: adjacent dst blocks (2b, 2b+1) share one padded slot run
    per chunk. Block 2b's edges fill forward from slot 0, block 2b+1's fill
    backward from the end; the boundary tile can hold edges of both blocks
    (their onehot columns mask each other out). This cuts gather padding from
    ~25% to ~13%, and gather descriptor generation on gpsimd is the kernel's
    rate limiter (~2 ns/desc with 4-queue overlap).
  - Scatter-add realized as PE matmul: for each 128-edge tile,
    aggT[fi, d] += G[e, fi]^T @ onehot[e, d], accumulated in PSUM over a
    128-dst-node block window.
  - Onehots for a whole block (all tiles incl. the self-loop tile) are built
    with TWO wide DVE tensor_tensor ops over [128, til*128] using broadcast
    access patterns whose innermost dim is a duplicated (stride 1, count 2)
    pair -- keeps the DVE 2x perf mode eligible.
  - Self-loop is a regular tile: G = own x rows (contiguous DMA), onehot =
    diag(selfnorm) from the same batched build.
  - Stage 2 per block: emb = aggT^T @ W + b (bias seeded via K=1 outer
    product), pipelined one block behind aggregation; outputs staged bf16.
  - Groups of 4 pairs (8 blocks) pipeline gather->onehot->matmul->stage2; the
    final group is a single pair so the drain tail is short.

Host does the O(E) graph prep with numpy; a pure-numpy emulation of the
device layout (_host_verify) can be enabled with GCN_HOST_VERIFY=1.
"""

import os
import numpy as np
import ml_dtypes

import concourse.bass as bass
import concourse.tile as tile
from concourse import bacc, mybir
from concourse.bass_utils import run_bass_kernel_spmd

P = 128            # partitions / tile edge
F = 128            # feature dim (in == out)
NC = 8             # cores
N = 100000         # nodes (full problem)
BLOCKS_PER_CORE = 98
NCHUNK = 4         # src chunks (int16 index range)
PAIRS_PER_GROUP = 4

BF16 = mybir.dt.bfloat16
F32 = mybir.dt.float32
I16 = mybir.dt.int16

_cache: dict = {}


def _wrap16(flat):
    """flat [n] int16 -> [32, n/16]: idx i at [i%16, i//16], tiled x2.

    dma_gather on queue q reads its indices only from partition band
    [32q, 32q+32) (one 16-partition wrap per Q7 core of the pair), so a
    group's four chunk-calls can share the same columns in different bands.
    """
    n = flat.shape[0]
    a = flat.reshape(n // 16, 16).T            # [16, n/16]
    return np.tile(a, (2, 1))


def _host_prep(x, W, b, edge_index, edge_weight, n_nodes, blocks_per_core,
               n_cores, n_chunks=NCHUNK):
    """Build per-core input maps. Returns (in_maps, layout)."""
    p = P
    npc = blocks_per_core * p
    n_pad = n_cores * npc
    cs = n_pad // n_chunks          # chunk rows
    assert cs < 32768
    n_blocks = n_cores * blocks_per_core
    npair = blocks_per_core // 2

    src = edge_index[0].astype(np.int64)
    dst = edge_index[1].astype(np.int64)
    w = edge_weight.astype(np.float64)

    deg = np.bincount(dst, weights=w, minlength=n_nodes) + 1.0  # + self-loop
    dinv = 1.0 / np.sqrt(deg)
    norm = dinv[src] * w * dinv[dst]

    blk = dst // p                            # global dst block
    chunk = src // cs                         # src chunk
    seg = blk * n_chunks + chunk              # segment id
    order = np.argsort(seg, kind="stable")
    n_segs = n_blocks * n_chunks
    cnt = np.bincount(seg[order], minlength=n_segs).reshape(n_blocks, n_chunks)
    starts = np.zeros(n_segs + 1, dtype=np.int64)
    starts[1:] = np.cumsum(cnt.ravel())

    Tq = max(1, int(np.ceil(cnt.max() / p)))
    til = n_chunks * Tq + 1                   # onehot cols per block (+self)

    # per-core counts [cores, bpc, nchunk] -> pair totals -> static T table
    cntc = cnt.reshape(n_cores, blocks_per_core, n_chunks)
    ptot = cntc[:, 0::2, :] + cntc[:, 1::2, :]        # [cores, npair, nchunk]
    T = np.maximum(Tq, np.ceil(ptot.max(axis=0) / p).astype(np.int64))
    # group structure: groups of PAIRS_PER_GROUP pairs with a gradual taper
    # (3,3,2,1) at the end: PE starts a group only after the whole group's
    # gather DMA lands, so small final groups shrink the drain tail, and the
    # gradual cadence step avoids the gather sprinting into the gather-pool
    # buffer cap (which showed as an 11us gpsimd stall at a 4->2 transition)
    groups = []
    i = 0
    while npair - i > 5:
        groups.append(list(range(i, i + PAIRS_PER_GROUP)))
        i += PAIRS_PER_GROUP
    while npair - i > 0:
        take = min(2, npair - i)
        groups.append(list(range(i, i + take)))
        i += take

    # self-loop norm (per global padded node)
    selfnorm = np.zeros(n_pad)
    selfnorm[:n_nodes] = dinv * dinv

    per_edge = {
        "lidx": src - chunk * cs,             # chunk-local src row
        "norm": norm,
        "dstl": dst % p,
    }

    x_pad = np.zeros((n_pad, F), dtype=ml_dtypes.bfloat16)
    x_pad[:n_nodes] = x.astype(ml_dtypes.bfloat16)
    iota = np.broadcast_to(np.arange(p, dtype=np.float64), (p, p))
    iota = np.ascontiguousarray(iota.astype(ml_dtypes.bfloat16))
    w_f32 = np.ascontiguousarray(W.astype(np.float32))
    b_f32 = np.ascontiguousarray(b.astype(np.float32).reshape(1, F))
    ones = np.ones((1, p), dtype=np.float32)

    in_maps = []
    for c in range(n_cores):
        b0 = c * blocks_per_core
        norm_meta = np.zeros((blocks_per_core, til, p))
        dstl_meta = np.zeros((blocks_per_core, til, p))
        norm_meta[:, n_chunks * Tq, :] = \
            selfnorm.reshape(n_blocks, p)[b0:b0 + blocks_per_core]
        dstl_meta[:, n_chunks * Tq, :] = np.arange(p)

        idx_groups = []    # per group: [128, maxcols] int16, band q = call q
        for g_pairs in groups:
            calls = []
            for q in range(n_chunks):
                parts = []
                for pp in g_pairs:
                    bA = 2 * pp
                    bB = 2 * pp + 1
                    S = int(T[pp, q]) * p
                    sl_idx = np.zeros(S, dtype=np.int16)
                    for bb, fwd in ((bA, True), (bB, False)):
                        gb = b0 + bb
                        s0, s1 = starts[gb * n_chunks + q], \
                            starts[gb * n_chunks + q + 1]
                        eid = order[s0:s1]
                        nE = s1 - s0
                        off = 0 if fwd else S - nE
                        sl_idx[off:off + nE] = per_edge["lidx"][eid]
                        # onehot meta columns for this block & chunk
                        nloc = np.zeros(Tq * p)
                        dloc = np.zeros(Tq * p)
                        moff = off if fwd else off - (int(T[pp, q]) - Tq) * p
                        nloc[moff:moff + nE] = per_edge["norm"][eid]
                        dloc[moff:moff + nE] = per_edge["dstl"][eid]
                        norm_meta[bb, q * Tq:(q + 1) * Tq, :] = \
                            nloc.reshape(Tq, p)
                        dstl_meta[bb, q * Tq:(q + 1) * Tq, :] = \
                            dloc.reshape(Tq, p)
                    parts.append(sl_idx)
                calls.append(_wrap16(np.concatenate(parts)))
            maxcols = max(ca.shape[1] for ca in calls)
            arr = np.zeros((P, maxcols), dtype=np.int16)
            for q, ca in enumerate(calls):
                arr[32 * q:32 * (q + 1), :ca.shape[1]] = ca
            idx_groups.append(np.ascontiguousarray(arr))

        def meta_layout(m):
            # [bpc, til, p] -> [p, bpc*til] -> paired duplication (x2 cols)
            flat = m.transpose(2, 0, 1).reshape(p, blocks_per_core * til)
            return np.ascontiguousarray(
                np.repeat(flat, 2, axis=1).astype(ml_dtypes.bfloat16))

        im = {
            "x": x_pad,
            "xself": np.ascontiguousarray(x_pad[c * npc:(c + 1) * npc]),
            "w_in": w_f32,
            "b_in": b_f32,
            "ones_in": ones,
            "iota_in": iota,
            "norm_in": meta_layout(norm_meta),
            "dstl_in": meta_layout(dstl_meta),
        }
        for gi, arr in enumerate(idx_groups):
            im[f"idx{gi}_in"] = arr
        in_maps.append(im)

    layout = {
        "Tq": Tq, "til": til, "T": tuple(map(tuple, T)),
        "groups": tuple(map(tuple, groups)),
        "n_pad": n_pad, "bpc": blocks_per_core, "n_chunks": n_chunks,
    }
    return in_maps, layout


def _host_verify(in_maps, layout, W, b, exp_emb, core=0):
    """Numpy emulation of the device compute path for one core."""
    p = P
    Tq, til = layout["Tq"], layout["til"]
    T = np.array(layout["T"])
    groups = layout["groups"]
    n_chunks = layout["n_chunks"]
    cs = layout["n_pad"] // n_chunks
    bpc = layout["bpc"]
    im = in_maps[core]
    xf = np.asarray(im["x"]).astype(np.float64)
    xself = np.asarray(im["xself"]).astype(np.float64)
    nm = np.asarray(im["norm_in"]).astype(np.float64)[:, ::2]   # unpair
    dm = np.asarray(im["dstl_in"]).astype(np.float64)[:, ::2]
    aggT = np.zeros((bpc, p, F))
    for gi, g_pairs in enumerate(groups):
        idx = np.asarray(im[f"idx{gi}_in"])            # [128, maxcols]
        for q in range(n_chunks):
            ncols = int(T[list(g_pairs), q].sum()) * p // 16
            band = idx[32 * q:32 * q + 16]
            flat = band[:, :ncols].T.reshape(-1)       # unwrap
            poff = 0
            for pp in g_pairs:
                S = int(T[pp, q]) * p
                sl = flat[poff:poff + S].astype(np.int64)
                poff += S
                for half, base in ((0, 0), (1, int(T[pp, q]) - Tq)):
                    bb = 2 * pp + half
                    for t in range(Tq):
                        G = xf[q * cs + sl[(base + t) * p:(base + t + 1) * p]]
                        col = bb * til + q * Tq + t
                        oh = (np.arange(p)[None, :] == dm[:, col][:, None]) \
                            * nm[:, col][:, None]
                        aggT[bb] += oh.T @ G
    errs = []
    for bb in range(bpc):
        sn = nm[:, bb * til + n_chunks * Tq]
        agg = aggT[bb] + xself[bb * p:(bb + 1) * p] * sn[:, None]
        emb = agg @ W.astype(np.float64) + b.astype(np.float64)
        r0 = (core * bpc + bb) * p
        ref = exp_emb[r0:r0 + p]
        if len(ref):
            errs.append(np.abs(emb[:len(ref)] - ref).max())
    print(f"host_verify core {core}: max err {max(errs):.4e}")
    return max(errs)


def _build_program(layout):
    """Emit the SPMD Tile program. Same program runs on every core."""
    p = P
    Tq = layout["Tq"]
    til = layout["til"]
    T = np.array(layout["T"])
    groups = layout["groups"]
    blocks_per_core = layout["bpc"]
    n_chunks = layout["n_chunks"]
    n_pad = layout["n_pad"]
    npc = blocks_per_core * p
    cs = n_pad // n_chunks

    # per-(group, chunk) call sizes in tiles
    call_tiles = [[int(T[list(g), q].sum()) for q in range(n_chunks)]
                  for g in groups]
    max_tiles_q = [max(ct[q] for ct in call_tiles) for q in range(n_chunks)]
    max_blocks = max(2 * len(g) for g in groups)

    nc = bacc.Bacc("TRN2", target_bir_lowering=False, debug=False,
                   enable_asserts=False, num_devices=NC,
                   num_swdge_queues=4)

    x_d = nc.dram_tensor("x", [n_pad, F], BF16, kind="ExternalInput")
    xself_d = nc.dram_tensor("xself", [npc, F], BF16, kind="ExternalInput")
    w_d = nc.dram_tensor("w_in", [F, F], F32, kind="ExternalInput")
    b_d = nc.dram_tensor("b_in", [1, F], F32, kind="ExternalInput")
    ones_d = nc.dram_tensor("ones_in", [1, p], F32, kind="ExternalInput")
    iota_d = nc.dram_tensor("iota_in", [p, p], BF16, kind="ExternalInput")
    norm_d = nc.dram_tensor("norm_in", [p, blocks_per_core * til * 2], BF16,
                            kind="ExternalInput")
    dstl_d = nc.dram_tensor("dstl_in", [p, blocks_per_core * til * 2], BF16,
                            kind="ExternalInput")
    idx_d = []
    for gi, ct in enumerate(call_tiles):
        cols = max(ct) * p // 16          # calls stacked in partition bands
        idx_d.append(nc.dram_tensor(f"idx{gi}_in", [p, cols], I16,
                                    kind="ExternalInput"))
    emb_d = nc.dram_tensor("emb_out", [npc, F], BF16, kind="ExternalOutput")
    relu_d = nc.dram_tensor("relu_out", [npc, F], BF16, kind="ExternalOutput")

    emb_v = emb_d.ap().rearrange("(B q) f -> q B f", q=p)    # [p, blocks, F]
    relu_v = relu_d.ap().rearrange("(B q) f -> q B f", q=p)
    xself_v = xself_d.ap().rearrange("(B q) f -> q B f", q=p)

    with tile.TileContext(nc) as tc:
        with (
            tc.tile_pool(name="const", bufs=1) as const_pool,
            tc.tile_pool(name="gather", bufs=3) as gpool,
            tc.tile_pool(name="onehot", bufs=10) as ohpool,
            tc.tile_pool(name="aggsb", bufs=3) as aggpool,
            tc.tile_pool(name="outsb", bufs=2) as outpool,
            tc.tile_pool(name="psum_agg", bufs=3, space="PSUM") as ps_agg,
            tc.tile_pool(name="psum_emb", bufs=3, space="PSUM") as ps_emb,
        ):
            idx_g = []
            for gi, ct in enumerate(call_tiles):
                cols = max(ct) * p // 16
                t = const_pool.tile([p, cols], I16, name=f"idx_g{gi}")
                idx_g.append(t)
            nc.sync.dma_start(out=idx_g[0][:], in_=idx_d[0].ap())
            w_sb = const_pool.tile([F, F], F32)
            nc.sync.dma_start(out=w_sb[:], in_=w_d.ap())
            b_sb = const_pool.tile([1, F], F32)
            nc.sync.dma_start(out=b_sb[:], in_=b_d.ap())
            ones_sb = const_pool.tile([1, p], F32)
            nc.sync.dma_start(out=ones_sb[:], in_=ones_d.ap())
            iota_sb = const_pool.tile([p, p], BF16)
            nc.sync.dma_start(out=iota_sb[:], in_=iota_d.ap())
            norm_sb = const_pool.tile([p, blocks_per_core * til * 2], BF16)
            nc.sync.dma_start(out=norm_sb[:], in_=norm_d.ap())
            dstl_sb = const_pool.tile([p, blocks_per_core * til * 2], BF16)
            nc.sync.dma_start(out=dstl_sb[:], in_=dstl_d.ap())
            for gi in range(1, len(groups)):
                nc.sync.dma_start(out=idx_g[gi][:], in_=idx_d[gi].ap())

            iota_bc = (iota_sb[:].rearrange("q (a f b) -> q a f b", a=1, b=2)
                       .to_broadcast([p, til, p // 2, 2]))

            pending = []     # deferred stage-2 closures (one block behind)

            def flush_pending():
                while pending:
                    pending.pop(0)()

            for gi, g_pairs in enumerate(groups):
                nblk = 2 * len(g_pairs)
                blk0 = 2 * g_pairs[0]
                gq = []
                for q in range(n_chunks):
                    ntile = call_tiles[gi][q]
                    gt = gpool.tile([p, max_tiles_q[q] * F], BF16, tag=f"g{q}")
                    nidx = ntile * p
                    nc.gpsimd.dma_gather(
                        out_ap=gt[:, :ntile * F]
                        .rearrange("q (j f) -> q j f", f=F),
                        in_ap=x_d.ap()[q * cs:(q + 1) * cs, :],
                        idxs_ap=idx_g[gi][:, :nidx // 16],
                        num_idxs=nidx,
                        num_idxs_reg=nidx,
                        elem_size=F,
                        single_packet=False,
                        queue_num=q)
                    gq.append(gt)
                gs = gpool.tile([p, max_blocks * F], BF16, tag="gself")
                nc.sync.dma_start(
                    out=gs[:, :nblk * F].rearrange("q (B f) -> q B f", f=F),
                    in_=xself_v[:, blk0:blk0 + nblk, :])

                emb_st = outpool.tile([p, max_blocks * F], BF16, tag="emb_st")
                relu_st = outpool.tile([p, max_blocks * F], BF16,
                                       tag="relu_st")
                for j, pp in enumerate(g_pairs):
                    # tile offset of this pair within each chunk's call
                    pair_off = [int(T[list(g_pairs[:j]), q].sum())
                                for q in range(n_chunks)]
                    for half in range(2):
                        bb = 2 * pp + half
                        bi = 2 * j + half
                        c0 = bb * til
                        oh = ohpool.tile([p, til * p], BF16, tag="oh")
                        oh_v = oh[:].rearrange("q (u f b) -> q u f b",
                                               f=p // 2, b=2)
                        dstl_bc = (dstl_sb[:, 2 * c0:2 * (c0 + til)]
                                   .rearrange("q (u a b) -> q u a b",
                                              a=1, b=2)
                                   .to_broadcast([p, til, p // 2, 2]))
                        nc.vector.tensor_tensor(
                            out=oh_v, in0=iota_bc, in1=dstl_bc,
                            op=mybir.AluOpType.is_equal)
                        norm_bc = (norm_sb[:, 2 * c0:2 * (c0 + til)]
                                   .rearrange("q (u a b) -> q u a b",
                                              a=1, b=2)
                                   .to_broadcast([p, til, p // 2, 2]))
                        nc.vector.tensor_tensor(
                            out=oh_v, in0=oh_v, in1=norm_bc,
                            op=mybir.AluOpType.mult)

                        agg_ps = ps_agg.tile([p, p], F32)
                        for u in range(til - 1):
                            q, t = divmod(u, Tq)
                            base = pair_off[q] + (0 if half == 0 else
                                                  int(T[pp, q]) - Tq)
                            nc.tensor.matmul(
                                out=agg_ps[:],
                                lhsT=gq[q][:, (base + t) * F:
                                           (base + t + 1) * F],
                                rhs=oh[:, u * p:(u + 1) * p],
                                start=(u == 0), stop=False)
                        nc.tensor.matmul(
                            out=agg_ps[:],
                            lhsT=gs[:, bi * F:(bi + 1) * F],
                            rhs=oh[:, (til - 1) * p:til * p],
                            start=False, stop=True)

                        agg_sb = aggpool.tile([p, p], F32, tag="agg")
                        nc.scalar.activation(
                            out=agg_sb[:], in_=agg_ps[:],
                            func=mybir.ActivationFunctionType.Copy)

                        def stage2(agg_sb=agg_sb, emb_st=emb_st,
                                   relu_st=relu_st, bi=bi, nblk=nblk,
                                   blk0=blk0):
                            emb_ps = ps_emb.tile([p, F], F32)
                            nc.tensor.matmul(out=emb_ps[:], lhsT=ones_sb[:],
                                             rhs=b_sb[:], start=True,
                                             stop=False)
                            nc.tensor.matmul(out=emb_ps[:], lhsT=agg_sb[:],
                                             rhs=w_sb[:], start=False,
                                             stop=True)
                            nc.scalar.activation(
                                out=emb_st[:, bi * F:(bi + 1) * F],
                                in_=emb_ps[:],
                                func=mybir.ActivationFunctionType.Copy)
                            nc.scalar.activation(
                                out=relu_st[:, bi * F:(bi + 1) * F],
                                in_=emb_ps[:],
                                func=mybir.ActivationFunctionType.Relu)
                            if bi == nblk - 1:
                                nc.sync.dma_start(
                                    out=emb_v[:, blk0:blk0 + nblk, :],
                                    in_=emb_st[:, :nblk * F]
                                    .rearrange("q (B f) -> q B f", f=F))
                                nc.sync.dma_start(
                                    out=relu_v[:, blk0:blk0 + nblk, :],
                                    in_=relu_st[:, :nblk * F]
                                    .rearrange("q (B f) -> q B f", f=F))

                        flush_pending()
                        pending.append(stage2)
            flush_pending()

    nc.compile()
    return nc


def _get_program(layout):
    key = (layout["Tq"], layout["til"], layout["T"], layout["groups"],
           layout["n_pad"], layout["bpc"], layout["n_chunks"])
    if key not in _cache:
        _cache[key] = _build_program(layout)
    return _cache[key]


def run(x, W, b, edge_index, edge_weight, n_nodes, blocks_per_core, n_cores,
        n_chunks=NCHUNK, trace=False):
    in_maps, layout = _host_prep(x, W, b, edge_index, edge_weight,
                                 n_nodes, blocks_per_core, n_cores, n_chunks)
    nc = _get_program(layout)
    res = run_bass_kernel_spmd(nc, in_maps, list(range(n_cores)), trace=trace)
    emb = np.concatenate([np.asarray(res.results[c]["emb_out"])
                          for c in range(n_cores)], axis=0)[:n_nodes]
    relu = np.concatenate([np.asarray(res.results[c]["relu_out"])
                           for c in range(n_cores)], axis=0)[:n_nodes]
    return (emb.astype(np.float32), relu.astype(np.float32)), res


def kernel(x, W, b, level, edge_index, edge_weight):
    x = np.asarray(x)
    W = np.asarray(W)
    b = np.asarray(b)
    edge_index = np.asarray(edge_index)
    edge_weight = np.asarray(edge_weight)
    (emb, relu), _ = run(x, W, b, edge_index, edge_weight,
                         N, BLOCKS_PER_CORE, NC)
    return emb, relu
